# revision 22
# baseline (speedup 1.0000x reference)
"""Trainium2 Bass kernel for the logic-model log-likelihood (optimized).

Key structure (per core: SC=8 samples, ROWS=24 (s,p) rows):
  log-sum:  k_q = exp(-D*tq) * sum_e [tq-te>TOL] * (mask*exp(D*te))
    - tq broadcast to all partitions as FOUR bf16 chunks on the Act HWDGE
      queue (bf16 halves the ~3MB of DMA traffic; only the query side of
      the compare is quantized, +/-0.125 at t=50 -> rel err ~1.4e-3 vs
      the 2e-2 gate; the sync queue keeps just the input+output DMAs)
    - compare planes: DVE scalar_tensor_tensor with 0-stride-broadcast
      te operand (exact f32), one wave of 3 ops per broadcast chunk,
      c01/c02 emitted before the 2x-wide c0p so h0 matvecs start early
    - contraction: PE matvecs, compare plane stationary (bf16 FWL),
      a=mask*exp(D*te) column moving; feat PSUM [128 q, 24 (s,h)]
    - lam = b + w*exp(-D*tq)*feat; masked ln sum via stt accum_out
  integral: closed-form geometric sum per event (see baseline docstring),
    floor via round-nearest(u-0.5) int32 convert; runs entirely under the
    broadcast/compare window (DVE + Act only; Pool/GPSIMD is never used --
    its per-op ucode overhead is ~1us).
  final: two PSUM-accumulated [1,1] matmuls + copy + out DMA.

Engine discipline learned from measurement:
  - act ops ~500ns each, and act-table reloads (Sigmoid vs Exp/Ln) cost
    1.3us -> only Exp/Ln/Copy-free funcs; only 4 act ops total.
  - DVE ops ~60-100ns fixed + ~0.6ns/col -> everything else on DVE.
  - For_i iterations serialize -> minimize the serial critical path:
    bcast DMA -> compares -> matvecs -> epilogue -> out.

Sharding: data-parallel over samples S: 8 cores x 8 samples. Each core emits
a scalar partial; host sums the 8 partials (the gather/unshard step).
"""
import os
import sys

import ml_dtypes
import numpy as np

sys.path.insert(0, "/opt/trn_rl_repo")

import concourse.bacc as bacc
import concourse.mybir as mybir
from concourse import tile
from concourse.bass_utils import run_bass_kernel_spmd

F32 = mybir.dt.float32
I32 = mybir.dt.int32
BF16 = mybir.dt.bfloat16
AF = mybir.ActivationFunctionType
ALU = mybir.AluOpType

N_CORES = 8
S, P, E = 64, 3, 128
SC = S // N_CORES          # samples per core
ROWS = SC * P              # 24 (s,p) rows per core
DECAY, RES, TOL = 0.8, 0.03, 0.1
G = 1667                   # len(np.arange(0, 50, 0.03))
INV1MR = float(1.0 / (1.0 - np.exp(-DECAY * RES)))
BODY = np.array([[0, 1, 1], [1, 0, 0], [1, 0, 0]], dtype=np.float32)

# packed const block [128, cols]
_CB_ID56 = 0      # ident 56x56 (one-shot transpose of t+mask rows)
_CB_OC = 56       # ones column [128,1]
_CB_OC0 = 57      # ones column with 0 at partition 0 (skip eq=0 queries)
_CB_OR = 58       # ones row on partition 0 [1,128] (cols 58..186)
_CB_LM = 186      # lmat [3,24]  (BODY tiled: lmat[h, 3s+p] = BODY[h,p])
_CB_SEL = 210     # selm [3,24]  (selm[k, n] = 1 if k == n%3)
_CB_CV = 234      # const val columns
CVALS = [0.0, -DECAY * RES]
_CB_COLS = 236


def _const_block():
    cb = np.zeros((128, _CB_COLS), np.float32)
    cb[0:56, _CB_ID56:_CB_ID56 + 56] = np.eye(56)
    cb[:, _CB_OC] = 1.0
    cb[1:, _CB_OC0] = 1.0
    cb[0, _CB_OR:_CB_OR + 128] = 1.0
    cb[64:64 + P, _CB_LM:_CB_LM + ROWS] = np.tile(BODY, (1, SC))
    sel = np.zeros((P, ROWS), np.float32)
    for n in range(ROWS):
        sel[n % P, n] = 1.0
    cb[64:64 + P, _CB_SEL:_CB_SEL + ROWS] = sel
    cb[:, _CB_CV:_CB_CV + len(CVALS)] = np.array(CVALS, np.float32)
    return cb


def _build_nc(reps=1):
    nc = bacc.Bacc(None, target_bir_lowering=False)
    em_d = nc.dram_tensor("em", [68, E], F32, kind="ExternalInput")
    emh_d = nc.dram_tensor("emh", [ROWS, E], BF16, kind="ExternalInput")
    wb_d = nc.dram_tensor("wb", [P, 2], F32, kind="ExternalInput")
    out_d = nc.dram_tensor("out", [1, 1], F32, kind="ExternalOutput")
    cb_d = nc.inline_tensor(_const_block(), "cblock")

    with tile.TileContext(nc) as tc:
        with (
            tc.tile_pool(name="const", bufs=1) as cpool,
            tc.tile_pool(name="inp", bufs=2) as ipool,
            tc.tile_pool(name="tq", bufs=2) as tqpool,
            tc.tile_pool(name="cmp", bufs=2) as cppool,
            tc.tile_pool(name="small", bufs=2) as spool,
            tc.tile_pool(name="psT", bufs=1, space="PSUM") as psT,
            tc.tile_pool(name="psF", bufs=2, space="PSUM") as psF,
            tc.tile_pool(name="psS", bufs=1, space="PSUM") as psS,
            tc.tile_pool(name="psFin", bufs=1, space="PSUM") as psFin,
        ):
            # ---- one DMA for all constants ----
            cblk = cpool.tile([128, _CB_COLS], F32, tag="cblk")
            nc.sync.dma_start(cblk[:], cb_d[:])
            ident56 = cblk[0:56, _CB_ID56:_CB_ID56 + 56]
            ones_col = cblk[:, _CB_OC:_CB_OC + 1]
            ones0_col = cblk[:, _CB_OC0:_CB_OC0 + 1]
            ones_row = cblk[0:1, _CB_OR:_CB_OR + 128]
            lmat = cblk[64:64 + P, _CB_LM:_CB_LM + ROWS]
            selm = cblk[64:64 + P, _CB_SEL:_CB_SEL + ROWS]
            for ci, cval in enumerate(CVALS):
                nc.const_aps.aps[(F32, cval)] = cblk[:, _CB_CV + ci:_CB_CV + ci + 1]

            def _body():
                # ---- inputs; tq broadcast split over both HWDGE queues ----
                im = ipool.tile([68, E], F32, tag="im")
                nc.sync.dma_start(im[:], em_d[:])
                t_sb = im[0:ROWS, :]
                m_sb = im[32:32 + ROWS, :]
                emh_flat = emh_d[:].rearrange("a b -> (a b)")
                QTR = SC * P * E // 4  # 768 = 2 samples
                tq_all = tqpool.tile([128, SC * P * E], BF16, tag="tq_all")
                for c in range(4):
                    nc.scalar.dma_start(
                        tq_all[:, c * QTR:(c + 1) * QTR],
                        emh_flat[c * QTR:(c + 1) * QTR].partition_broadcast(128))
                wbt = im[64:67, 0:2]

                # ---- one transpose for t and mask: [56,128] -> [128,56] ----
                trm_ps = psT.tile([128, 56], F32, tag="trm")
                nc.tensor.transpose(trm_ps[:], im[0:56, :], ident56)
                trm = spool.tile([128, 56], F32, tag="trm_sb")
                nc.vector.tensor_copy(trm[:], trm_ps[:])
                t_T = trm[:, 0:ROWS]
                mask_T = trm[:, 32:32 + ROWS]

                # ---- derived per-event tiles ----
                expT = spool.tile([128, ROWS], F32, tag="expT")
                nc.scalar.activation(expT[:], t_T, AF.Exp, scale=DECAY)
                a_T = spool.tile([128, ROWS], BF16, tag="a_T")  # mask*exp(D*t)
                nc.vector.tensor_mul(a_T[:], expT[:], mask_T)
                eq_T = spool.tile([128, ROWS], F32, tag="eq_T")  # exp(-D*t)
                nc.vector.reciprocal(eq_T[:], expT[:])

                # ---- w24/b24 broadcast [128, 48] + v/b_col (PE) ----
                smalls = psS.tile([ROWS, 52], F32, tag="smalls")
                wbrow_ps = smalls[0:1, 0:2 * ROWS]
                v_ps = smalls[0:ROWS, 48:49]
                bcol_ps = smalls[0:ROWS, 49:50]
                nc.tensor.matmul(wbrow_ps[0:1, 0:ROWS], wbt[:, 0:1], selm,
                                 start=True, stop=True)
                nc.tensor.matmul(wbrow_ps[0:1, ROWS:2 * ROWS], wbt[:, 1:2],
                                 selm, start=True, stop=True)
                nc.tensor.matmul(v_ps[:], lmat, wbt[:, 0:1], start=True,
                                 stop=True)
                nc.tensor.matmul(bcol_ps[:], selm, wbt[:, 1:2], start=True,
                                 stop=True)
                wbrow = spool.tile([1, 2 * ROWS], F32, tag="wbrow_sb")
                nc.vector.tensor_copy(wbrow[:], wbrow_ps[0:1, :])
                wb24_ps = psT.tile([128, 2 * ROWS], F32, tag="wb24_ps")
                nc.tensor.matmul(wb24_ps[:], ones_row, wbrow[:], start=True,
                                 stop=True)
                wb24 = spool.tile([128, 2 * ROWS], F32, tag="wb24")
                nc.vector.tensor_copy(wb24[:], wb24_ps[:])
                w24 = wb24[:, 0:ROWS]
                b24 = wb24[:, ROWS:2 * ROWS]
                # ew = w * exp(-D*tq), ready before the compare window ends
                ew = spool.tile([128, ROWS], F32, tag="ew")
                nc.vector.tensor_mul(ew[:], w24, eq_T[:])

                # ---- integral (under the bcast/compare window) ----
                # C(t) = exp(D*t)*relu(exp(-D*RES*(floor((t+TOL)/RES)+1))
                #                      - exp(-D*G*RES))
                u_t = ipool.tile([ROWS, E], F32, tag="u_t")
                nc.vector.tensor_scalar(u_t[:], t_sb, TOL - 0.5 * RES,
                                        1.0 / RES, ALU.add, ALU.mult)
                ci_t = ipool.tile([ROWS, E], I32, tag="ci_t")
                nc.vector.tensor_copy(ci_t[:], u_t[:])
                cf_t = ipool.tile([ROWS, E], F32, tag="cf_t")
                nc.vector.tensor_copy(cf_t[:], ci_t[:])
                e_f = ipool.tile([ROWS, E], F32, tag="e_f")
                nc.scalar.activation(e_f[:], cf_t[:], AF.Exp,
                                     scale=-DECAY * RES, bias=-DECAY * RES)
                q1 = ipool.tile([ROWS, E], F32, tag="q1")
                nc.vector.tensor_scalar(q1[:], e_f[:],
                                        float(np.exp(-DECAY * G * RES)), 0.0,
                                        ALU.subtract, ALU.max)
                eDt = ipool.tile([56, E], F32, tag="eDt")
                nc.scalar.activation(eDt[32:56, :], t_sb, AF.Exp, scale=DECAY)
                am = ipool.tile([ROWS, E], F32, tag="am")
                nc.vector.tensor_mul(am[:], eDt[32:56, :], m_sb)
                cm = ipool.tile([ROWS, E], F32, tag="cm")
                kint = spool.tile([ROWS, 1], F32, tag="kint")
                nc.vector.scalar_tensor_tensor(
                    cm[:], q1[:], 1.0, am[:], ALU.mult, ALU.mult,
                    accum_out=kint[:])
                # x1 = (kint * -RES*INV1MR)*v ; x2 = (b_col * -G*RES) + x1
                x1 = spool.tile([ROWS, 1], F32, tag="x1")
                nc.vector.scalar_tensor_tensor(x1[:], kint[:], -RES * INV1MR,
                                               v_ps[:], ALU.mult, ALU.mult)
                x2 = spool.tile([ROWS, 1], F32, tag="x2")
                nc.vector.scalar_tensor_tensor(x2[:], bcol_ps[:], -RES * G,
                                               x1[:], ALU.mult, ALU.add)

                # ---- compare planes (DVE), chunked per broadcast half;
                # feat matvecs (PE) follow each chunk ----
                tcols = t_T.rearrange("p (s c) -> p s c", s=SC, c=P)
                tq3 = tq_all[:].rearrange("p (s x) -> p s x", s=SC, x=P * E)
                c01 = cppool.tile([128, SC * E], BF16, tag="c01")
                c01v = c01[:].rearrange("p (s e) -> p s e", s=SC, e=E)
                c02 = cppool.tile([128, SC * E], BF16, tag="c02")
                c02v = c02[:].rearrange("p (s e) -> p s e", s=SC, e=E)
                c0p = cppool.tile([128, SC * 2 * E], BF16, tag="c0p")
                c0pv = c0p[:].rearrange("p (s e) -> p s e", s=SC, e=2 * E)
                feat_ps = psF.tile([128, ROWS], F32, tag="feat")

                HS = SC // 4
                for half in range(4):
                    s0, s1_ = half * HS, (half + 1) * HS
                    nc.vector.scalar_tensor_tensor(
                        c01v[:, s0:s1_, :], tq3[:, s0:s1_, 0:E], TOL,
                        tcols[:, s0:s1_, 1:2].broadcast_to([128, HS, E]),
                        ALU.subtract, ALU.is_gt)
                    nc.vector.scalar_tensor_tensor(
                        c02v[:, s0:s1_, :], tq3[:, s0:s1_, 0:E], TOL,
                        tcols[:, s0:s1_, 2:3].broadcast_to([128, HS, E]),
                        ALU.subtract, ALU.is_gt)
                    nc.vector.scalar_tensor_tensor(
                        c0pv[:, s0:s1_, :], tq3[:, s0:s1_, E:P * E], TOL,
                        tcols[:, s0:s1_, 0:1].broadcast_to([128, HS, 2 * E]),
                        ALU.subtract, ALU.is_gt)
                    for s in range(s0, s1_):
                        nc.tensor.matmul(
                            feat_ps[:, s * P:s * P + 1],
                            c01[:, s * E:(s + 1) * E],
                            a_T[:, s * P + 1:s * P + 2],
                            start=True, stop=False)
                        nc.tensor.matmul(
                            feat_ps[:, s * P:s * P + 1],
                            c02[:, s * E:(s + 1) * E],
                            a_T[:, s * P + 2:s * P + 3],
                            start=False, stop=True)
                        nc.tensor.matmul(
                            feat_ps[:, s * P + 1:s * P + 2],
                            c0p[:, s * 2 * E:s * 2 * E + E],
                            a_T[:, s * P:s * P + 1], start=True, stop=True)
                        nc.tensor.matmul(
                            feat_ps[:, s * P + 2:s * P + 3],
                            c0p[:, s * 2 * E + E:(s + 1) * 2 * E],
                            a_T[:, s * P:s * P + 1], start=True, stop=True)

                # ---- epilogue: lam = b + w*exp(-D*tq)*feat; ln sum ----
                s2t = spool.tile([128, ROWS], F32, tag="s2")
                nc.vector.tensor_mul(s2t[:], feat_ps[:], ew[:])
                s3t = spool.tile([128, ROWS], F32, tag="s3")
                nc.vector.tensor_add(s3t[:], s2t[:], b24)
                lnv = spool.tile([128, ROWS], F32, tag="lnv")
                nc.scalar.activation(lnv[:], s3t[:], AF.Ln)
                lnm = spool.tile([128, ROWS], F32, tag="lnm")
                red = spool.tile([128, 1], F32, tag="red")
                nc.vector.scalar_tensor_tensor(
                    lnm[:], lnv[:], 1.0, mask_T, ALU.mult, ALU.mult,
                    accum_out=red[:])

                # ---- final: fin = sum_part(red over q>=1) + sum_part(x2) ----
                fin_ps = psFin.tile([1, 1], F32, tag="fin_ps")
                nc.tensor.matmul(fin_ps[:], red[:], ones0_col,
                                 start=True, stop=False)
                nc.tensor.matmul(fin_ps[:], x2[:], ones_col[0:ROWS, :],
                                 start=False, stop=True)
                fin = spool.tile([1, 1], F32, tag="fin")
                nc.vector.tensor_copy(fin[:], fin_ps[:])
                nc.sync.dma_start(out_d[:], fin[:])

            if reps == 1:
                _body()
            else:
                with tc.For_i(0, reps, 1):
                    _body()

    nc.compile()
    return nc


_NC = None


def _get_nc():
    global _NC
    if _NC is None:
        _NC = _build_nc()
    return _NC


def make_in_maps(event_times, event_mask, base, weight):
    et = np.ascontiguousarray(np.asarray(event_times, np.float32))
    mk = np.ascontiguousarray(np.asarray(event_mask, np.float32))
    wb = np.stack([np.asarray(weight, np.float32).reshape(P),
                   np.asarray(base, np.float32).reshape(P)], axis=1)
    in_maps = []
    for c in range(N_CORES):
        em = np.zeros((68, E), np.float32)
        em[0:ROWS] = et[c * SC:(c + 1) * SC].reshape(ROWS, E)
        em[64:67, 0:2] = wb
        em[32:32 + ROWS] = mk[c * SC:(c + 1) * SC].reshape(ROWS, E)
        emh = (et[c * SC:(c + 1) * SC].reshape(ROWS, E)
               .astype(ml_dtypes.bfloat16))
        in_maps.append({"em": em, "wb": np.zeros((P, 2), np.float32),
                        "emh": emh})
    return in_maps


LAST_RESULT = None


def kernel(event_times, event_mask, base, weight, T_max=50, _trace=False, **_):
    global LAST_RESULT
    nc = _get_nc()
    in_maps = make_in_maps(event_times, event_mask, base, weight)
    kwargs = {}
    if _trace:
        kwargs = dict(trace=True, trace_cores=list(range(N_CORES)))
    res = run_bass_kernel_spmd(nc, in_maps, core_ids=list(range(N_CORES)),
                               **kwargs)
    LAST_RESULT = res
    total = np.float32(0.0)
    for r in res.results:
        total += np.float32(r["out"][0, 0])
    return np.asarray(total, dtype=np.float32)


def run_timing(in_maps, reps, core_ids=None):
    nc = _build_nc(reps=reps)
    if core_ids is None:
        core_ids = list(range(N_CORES))
    return run_bass_kernel_spmd(nc, in_maps, core_ids=core_ids)
